# revision 2
# baseline (speedup 1.0000x reference)
"""Trainium2 Bass kernel: one DecoderRNN step (embedding lookup -> GRU cell ->
dot-product attention over 4096 encoder states -> 50257-way output projection
-> log_softmax), tensor-parallel across 8 NeuronCores.

Sharding:
  - embedding table: vocab-sharded (6283 rows/core), masked-gather + AllReduce
  - GRU: output-row sharded (128 h-dims/core), AllGather of h_new
  - attention: encoder-length sharded (512 rows/core), flash-style softmax
    merge via AllGather of (partial ctx, max, sumexp)
  - output projection: vocab-row sharded (6656 padded rows/core) in bf16,
    two-pass over the contraction so the h_new half of the matvec starts
    before attention finishes; log_softmax via AllGather of (max, sumexp)
"""
import numpy as np
from contextlib import ExitStack

import concourse.bass as bass
import concourse.bacc as bacc
import concourse.tile as tile
from concourse import mybir, bass_isa
from concourse.bass_utils import run_bass_kernel_spmd
from concourse.masks import make_identity

import ml_dtypes

H = 1024
V = 50257
L = 4096
C = 8                 # cores
S = 6656              # padded vocab rows per core (13*512)
ES = 6283             # embedding rows per core (8*6283 = 50264 >= V)
NT = 13               # 512-wide n tiles per core
LT = 4                # 128-row encoder tiles per core (512 rows)
F32 = mybir.dt.float32
BF16 = mybir.dt.bfloat16
I32 = mybir.dt.int32
AF = mybir.ActivationFunctionType
RG = [list(range(C))]

_CACHE = {}


def _build():
    nc = bacc.Bacc("TRN2", target_bir_lowering=False, debug=False,
                   num_devices=C)
    dt = nc.dram_tensor
    tok = dt("tok", [1, 1], I32, kind="ExternalInput").ap()
    vbase = dt("vbase", [1, 1], I32, kind="ExternalInput").ap()
    etab = dt("etab", [2 * ES, 512], F32, kind="ExternalInput").ap()
    wih = dt("wih", [128, 8, 3, 128], F32, kind="ExternalInput").ap()
    whh = dt("whh", [128, 8, 3, 128], F32, kind="ExternalInput").ap()
    bih = dt("bih", [128, 3], F32, kind="ExternalInput").ap()
    bhh = dt("bhh", [128, 3], F32, kind="ExternalInput").ap()
    hsh = dt("hsh", [128], F32, kind="ExternalInput").ap()
    h8in = dt("h8in", [8, 128], F32, kind="ExternalInput").ap()
    eh = dt("eh", [LT, 128, H], F32, kind="ExternalInput").ap()
    wt = dt("wt", [NT, 2, 128, 8, 512], BF16, kind="ExternalInput").ap()
    blay = dt("blay", [128, 52], F32, kind="ExternalInput").ap()

    o_logits = dt("o_logits", [S], F32, kind="ExternalOutput").ap()
    o_hnew = dt("o_hnew", [H], F32, kind="ExternalOutput").ap()

    with tile.TileContext(nc) as tc, ExitStack() as ctx:
        sb = ctx.enter_context(tc.tile_pool(name="sb", bufs=1))
        slabs = ctx.enter_context(tc.tile_pool(name="slabs", bufs=6))
        ps = ctx.enter_context(tc.tile_pool(name="ps", bufs=2, space="PSUM"))
        psacc = ctx.enter_context(tc.tile_pool(name="psacc", bufs=1, space="PSUM"))
        dram = ctx.enter_context(tc.tile_pool(name="dram", bufs=1, space="DRAM"))

        # ---------------- early loads (priority = program order) ----------
        wih_sb = sb.tile([128, 8, 3, 128], F32, name="wih_sb")
        nc.sync.dma_start(out=wih_sb[:], in_=wih[:])
        whh_sb = sb.tile([128, 8, 3, 128], F32, name="whh_sb")
        nc.sync.dma_start(out=whh_sb[:], in_=whh[:])
        bih_sb = sb.tile([128, 3], F32, name="bih_sb")
        nc.sync.dma_start(out=bih_sb[:], in_=bih[:])
        bhh_sb = sb.tile([128, 3], F32, name="bhh_sb")
        nc.sync.dma_start(out=bhh_sb[:], in_=bhh[:])
        hsh_sb = sb.tile([128, 1], F32, name="hsh_sb")
        nc.sync.dma_start(out=hsh_sb[:], in_=hsh[:, None])
        h8_sb = sb.tile([8, 128], F32, name="h8_sb")
        nc.sync.dma_start(out=h8_sb[:], in_=h8in[:])
        tok_sb = sb.tile([1, 1], I32, name="tok_sb")
        nc.sync.dma_start(out=tok_sb[:], in_=tok[:])
        vb_sb = sb.tile([1, 1], I32, name="vb_sb")
        nc.sync.dma_start(out=vb_sb[:], in_=vbase[:])
        eh_sb = sb.tile([128, LT, H], F32, name="eh_sb")
        for t in range(LT):
            nc.sync.dma_start(out=eh_sb[:, t, :], in_=eh[t])
        blay_sb = sb.tile([128, 52], F32, name="blay_sb")
        nc.sync.dma_start(out=blay_sb[:], in_=blay[:])

        ident = sb.tile([128, 128], F32, name="ident")
        make_identity(nc, ident[:])

        # ---------------- embedding gather + AllReduce --------------------
        u = sb.tile([1, 1], I32, name="u")
        nc.vector.tensor_tensor(out=u[:], in0=tok_sb[:], in1=vb_sb[:],
                                op=mybir.AluOpType.subtract)
        uc = sb.tile([1, 1], I32, name="uc")
        nc.vector.tensor_scalar_max(uc[:], u[:], 0)
        nc.vector.tensor_scalar_min(uc[:], uc[:], ES - 1)
        msk = sb.tile([1, 1], I32, name="msk")
        nc.vector.tensor_tensor(out=msk[:], in0=u[:], in1=uc[:],
                                op=mybir.AluOpType.is_equal)
        mskf = sb.tile([1, 1], F32, name="mskf")
        nc.vector.tensor_copy(out=mskf[:], in_=msk[:])
        uc2 = sb.tile([2, 1], I32, name="uc2")
        nc.gpsimd.partition_broadcast(uc2[:], uc[:])
        msk2 = sb.tile([2, 1], F32, name="msk2")
        nc.gpsimd.partition_broadcast(msk2[:], mskf[:])
        offs = sb.tile([2, 1], I32, name="offs")
        nc.gpsimd.iota(offs[:], pattern=[[0, 1]], base=0, channel_multiplier=1)
        two_u = sb.tile([2, 1], I32, name="two_u")
        nc.vector.tensor_scalar_mul(two_u[:], uc2[:], 2)
        nc.vector.tensor_add(out=offs[:], in0=offs[:], in1=two_u[:])
        x2 = sb.tile([2, 512], F32, name="x2")
        nc.gpsimd.indirect_dma_start(
            out=x2[:], out_offset=None, in_=etab[:],
            in_offset=bass.IndirectOffsetOnAxis(ap=offs[:, :1], axis=0))
        xm = sb.tile([2, 512], F32, name="xm")
        nc.vector.tensor_scalar_mul(xm[:], x2[:], msk2[:, :1])
        xdr = dram.tile([2, 512], F32, name="xdr")
        nc.sync.dma_start(out=xdr[:], in_=xm[:])
        xfull = dram.tile([H], F32, name="xfull", addr_space="Shared")
        nc.gpsimd.collective_compute(
            "AllReduce", mybir.AluOpType.add, replica_groups=RG,
            ins=[xdr[:].opt()], outs=[xfull[:].opt()])
        x8 = sb.tile([8, 128], F32, name="x8")
        nc.sync.dma_start(out=x8[:], in_=xfull[:].rearrange("(a b) -> a b", a=8))
        xT_ps = ps.tile([128, 8], F32, name="xT_ps", tag="tp")
        nc.tensor.transpose(out=xT_ps[:], in_=x8[:], identity=ident[:8, :8])
        xT2 = sb.tile([128, 8, 2], F32, name="xT2")
        nc.vector.tensor_copy(out=xT2[:, :, 0:1], in_=xT_ps[:, :, None])
        nc.vector.tensor_copy(out=xT2[:, :, 1:2], in_=xT_ps[:, :, None])
        hT_ps = ps.tile([128, 8], F32, name="hT_ps", tag="tp")
        nc.tensor.transpose(out=hT_ps[:], in_=h8_sb[:], identity=ident[:8, :8])
        hT2 = sb.tile([128, 8, 2], F32, name="hT2")
        nc.vector.tensor_copy(out=hT2[:, :, 0:1], in_=hT_ps[:, :, None])
        nc.vector.tensor_copy(out=hT2[:, :, 1:2], in_=hT_ps[:, :, None])

        # ---------------- GRU (rows-on-partitions, N=2 padded) ------------
        g_ps = psacc.tile([128, 6, 2], F32, name="g_ps")
        for mat, (wsb, rhs2) in enumerate(((wih_sb, xT2), (whh_sb, hT2))):
            for j in range(3):
                for k in range(8):
                    nc.tensor.matmul(out=g_ps[:, mat * 3 + j, :],
                                     lhsT=wsb[:, k, j, :],
                                     rhs=rhs2[:, k, :],
                                     start=(k == 0), stop=(k == 7))
        gi_sb = sb.tile([128, 3], F32, name="gi_sb")
        nc.vector.tensor_add(out=gi_sb[:], in0=g_ps[:, 0:3, 0], in1=bih_sb[:])
        gh_sb = sb.tile([128, 3], F32, name="gh_sb")
        nc.vector.tensor_add(out=gh_sb[:], in0=g_ps[:, 3:6, 0], in1=bhh_sb[:])
        rz_in = sb.tile([128, 2], F32, name="rz_in")
        nc.vector.tensor_add(out=rz_in[:], in0=gi_sb[:, 0:2], in1=gh_sb[:, 0:2])
        rz = sb.tile([128, 2], F32, name="rz")
        nc.scalar.activation(out=rz[:], in_=rz_in[:], func=AF.Sigmoid)
        t1 = sb.tile([128, 1], F32, name="t1")
        nc.vector.tensor_mul(out=t1[:], in0=rz[:, 0:1], in1=gh_sb[:, 2:3])
        ngate = sb.tile([128, 1], F32, name="ngate")
        nc.scalar.activation(out=ngate[:], in_=gi_sb[:, 2:3], func=AF.Tanh,
                             bias=t1[:, :1])
        t2 = sb.tile([128, 1], F32, name="t2")
        nc.vector.tensor_tensor(out=t2[:], in0=hsh_sb[:], in1=ngate[:],
                                op=mybir.AluOpType.subtract)
        t3 = sb.tile([128, 1], F32, name="t3")
        nc.vector.tensor_mul(out=t3[:], in0=rz[:, 1:2], in1=t2[:])
        hnew = sb.tile([128, 1], F32, name="hnew")
        nc.vector.tensor_add(out=hnew[:], in0=ngate[:], in1=t3[:])
        hn_dr = dram.tile([128], F32, name="hn_dr")
        nc.sync.dma_start(out=hn_dr[:, None], in_=hnew[:])
        hfull = dram.tile([H], F32, name="hfull", addr_space="Shared")
        nc.gpsimd.collective_compute(
            "AllGather", mybir.AluOpType.bypass, replica_groups=RG,
            ins=[hn_dr[:].opt()], outs=[hfull[:].opt()])
        nc.sync.dma_start(out=o_hnew[:], in_=hfull[:])

        hf8 = sb.tile([8, 128], F32, name="hf8")
        nc.sync.dma_start(out=hf8[:], in_=hfull[:].rearrange("(a b) -> a b", a=8))
        hTn_ps = ps.tile([128, 8], F32, name="hTn_ps", tag="tp")
        nc.tensor.transpose(out=hTn_ps[:], in_=hf8[:], identity=ident[:8, :8])
        l1 = sb.tile([128, 8, 32], BF16, name="l1")
        nc.vector.memset(l1[:], 0.0)
        nc.vector.tensor_copy(out=l1[:, :, 0:1], in_=hTn_ps[:, :, None])

        h_rep = sb.tile([128, H], F32, name="h_rep")
        h_bc = bass.AP(tensor=hfull[:].tensor, offset=hfull[:].offset,
                       ap=[[0, 128], [1, H]])
        nc.gpsimd.dma_start(out=h_rep[:], in_=h_bc)

        # ---------------- big matvec pass 1 (h_new half, k=8..15) ---------
        logits_sb = sb.tile([1, S], F32, name="logits_sb")
        for n in range(NT):
            slab = slabs.tile([128, 8, 512], BF16, name="slab1")
            nc.sync.dma_start(out=slab[:], in_=wt[n, 1])
            pl = ps.tile([32, 512], F32, name="pl1", tag="pl")
            for k in range(8):
                nc.tensor.matmul(out=pl[:], lhsT=l1[:, k, :], rhs=slab[:, k, :],
                                 start=(k == 0), stop=(k == 7))
            nc.vector.tensor_copy(out=logits_sb[0:1, n * 512:(n + 1) * 512],
                                  in_=pl[0:1, :])

        # ---------------- attention (L-sharded) ---------------------------
        s_sc = sb.tile([128, LT], F32, name="s_sc")
        prod = sb.tile([128, H], F32, name="prod")
        for t in range(LT):
            nc.vector.tensor_mul(out=prod[:], in0=eh_sb[:, t, :], in1=h_rep[:])
            nc.vector.reduce_sum(out=s_sc[:, t:t + 1], in_=prod[:],
                                 axis=mybir.AxisListType.X)
        mloc = sb.tile([128, 1], F32, name="mloc")
        nc.vector.reduce_max(out=mloc[:], in_=s_sc[:], axis=mybir.AxisListType.X)
        mrep = sb.tile([128, 1], F32, name="mrep")
        nc.gpsimd.partition_all_reduce(mrep[:], mloc[:], channels=128,
                                       reduce_op=bass_isa.ReduceOp.max)
        negm = sb.tile([128, 1], F32, name="negm")
        nc.vector.tensor_scalar_mul(negm[:], mrep[:], -1.0)
        e_sc = sb.tile([128, LT], F32, name="e_sc")
        ecol = sb.tile([128, 1], F32, name="ecol")
        nc.scalar.activation(out=e_sc[:], in_=s_sc[:], func=AF.Exp,
                             bias=negm[:, :1], accum_out=ecol[:, :1])
        zrep = sb.tile([128, 1], F32, name="zrep")
        nc.gpsimd.partition_all_reduce(zrep[:], ecol[:], channels=128,
                                       reduce_op=bass_isa.ReduceOp.add)
        pc = psacc.tile([2, 2, 512], F32, name="pc", tag="ctxacc")
        e2 = sb.tile([128, LT, 2], F32, name="e2")
        nc.vector.tensor_copy(out=e2[:, :, 0:1], in_=e_sc[:, :, None])
        nc.vector.tensor_copy(out=e2[:, :, 1:2], in_=e_sc[:, :, None])
        for t in range(LT):
            for half in range(2):
                nc.tensor.matmul(out=pc[:, half, :], lhsT=e2[:, t, :],
                                 rhs=eh_sb[:, t, half * 512:(half + 1) * 512],
                                 start=(t == 0), stop=(t == LT - 1))
        pay = sb.tile([1, 1028], F32, name="pay")
        nc.vector.tensor_copy(out=pay[0:1, 0:1024],
                              in_=pc[0:1].rearrange("a b c -> a (b c)"))
        nc.vector.tensor_copy(out=pay[0:1, 1024:1025], in_=mrep[0:1, :])
        nc.vector.tensor_copy(out=pay[0:1, 1025:1026], in_=zrep[0:1, :])
        nc.vector.memset(pay[0:1, 1026:1028], 0.0)
        pay_dr = dram.tile([1028], F32, name="pay_dr")
        nc.sync.dma_start(out=pay_dr[:, None].rearrange("a b -> b a"), in_=pay[:])
        pay8_dr = dram.tile([C * 1028], F32, name="pay8_dr", addr_space="Shared")
        nc.gpsimd.collective_compute(
            "AllGather", mybir.AluOpType.bypass, replica_groups=RG,
            ins=[pay_dr[:].opt()], outs=[pay8_dr[:].opt()])
        pay8 = sb.tile([8, 1028], F32, name="pay8")
        nc.sync.dma_start(out=pay8[:],
                          in_=pay8_dr[:].rearrange("(a b) -> a b", a=8))
        m8 = pay8[:, 1024:1025]
        z8 = pay8[:, 1025:1026]
        mg8 = sb.tile([8, 1], F32, name="mg8")
        nc.gpsimd.partition_all_reduce(mg8[:], m8, channels=8,
                                       reduce_op=bass_isa.ReduceOp.max)
        negg = sb.tile([8, 1], F32, name="negg")
        nc.vector.tensor_scalar_mul(negg[:], mg8[:], -1.0)
        t8 = sb.tile([8, 1], F32, name="t8")
        nc.scalar.activation(out=t8[:], in_=m8, func=AF.Exp, bias=negg[:, :1])
        u8 = sb.tile([8, 1], F32, name="u8")
        nc.vector.tensor_mul(out=u8[:], in0=z8, in1=t8[:])
        zg8 = sb.tile([8, 1], F32, name="zg8")
        nc.gpsimd.partition_all_reduce(zg8[:], u8[:], channels=8,
                                       reduce_op=bass_isa.ReduceOp.add)
        scaled = sb.tile([8, 1024], F32, name="scaled")
        nc.vector.tensor_scalar_mul(scaled[:], pay8[:, 0:1024], t8[:, :1])
        ones82 = sb.tile([8, 2], F32, name="ones82")
        nc.vector.memset(ones82[:], 1.0)
        pctx = psacc.tile([2, 2, 512], F32, name="pctx", tag="ctxacc")
        for half in range(2):
            nc.tensor.matmul(out=pctx[:, half, :], lhsT=ones82[:],
                             rhs=scaled[:, half * 512:(half + 1) * 512],
                             start=True, stop=True)
        rec8 = sb.tile([8, 1], F32, name="rec8")
        nc.vector.reciprocal(out=rec8[:], in_=zg8[:])
        ctxn = sb.tile([1, 1024], F32, name="ctxn")
        nc.vector.tensor_scalar_mul(ctxn[:],
                                    pctx[0:1].rearrange("a b c -> a (b c)"),
                                    rec8[0:1, :1])
        ctx_dr = dram.tile([1024], F32, name="ctx_dr")
        nc.sync.dma_start(out=ctx_dr[None, :], in_=ctxn[:])
        c8 = sb.tile([8, 128], F32, name="c8")
        nc.sync.dma_start(out=c8[:],
                          in_=ctx_dr[:].rearrange("(a b) -> a b", a=8))
        cT_ps = ps.tile([128, 8], F32, name="cT_ps", tag="tp")
        nc.tensor.transpose(out=cT_ps[:], in_=c8[:], identity=ident[:8, :8])
        l0 = sb.tile([128, 8, 32], BF16, name="l0")
        nc.vector.memset(l0[:], 0.0)
        nc.vector.tensor_copy(out=l0[:, :, 0:1], in_=cT_ps[:, :, None])

        # ---------------- big matvec pass 2 (ctx half, k=0..7) ------------
        for n in range(NT):
            slab = slabs.tile([128, 8, 512], BF16, name="slab0")
            nc.sync.dma_start(out=slab[:], in_=wt[n, 0])
            pl = ps.tile([32, 512], F32, name="pl0", tag="pl")
            for k in range(8):
                nc.tensor.matmul(out=pl[:], lhsT=l0[:, k, :], rhs=slab[:, k, :],
                                 start=(k == 0), stop=(k == 7))
            nsl = slice(n * 512, (n + 1) * 512)
            nc.vector.tensor_add(out=logits_sb[0:1, nsl],
                                 in0=logits_sb[0:1, nsl], in1=pl[0:1, :])

        # ---------------- epilogue: log_softmax ---------------------------
        lg_dr = dram.tile([S], F32, name="lg_dr")
        nc.sync.dma_start(out=lg_dr[None, :], in_=logits_sb[:])
        lg = sb.tile([128, 52], F32, name="lg")
        nc.sync.dma_start(out=lg[:], in_=lg_dr[:].rearrange("(p f) -> p f", p=128))
        lgb = sb.tile([128, 52], F32, name="lgb")
        nc.vector.tensor_add(out=lgb[:], in0=lg[:], in1=blay_sb[:])
        m2l = sb.tile([128, 1], F32, name="m2l")
        nc.vector.reduce_max(out=m2l[:], in_=lgb[:], axis=mybir.AxisListType.X)
        m2r = sb.tile([128, 1], F32, name="m2r")
        nc.gpsimd.partition_all_reduce(m2r[:], m2l[:], channels=128,
                                       reduce_op=bass_isa.ReduceOp.max)
        negm2 = sb.tile([128, 1], F32, name="negm2")
        nc.vector.tensor_scalar_mul(negm2[:], m2r[:], -1.0)
        e2t = sb.tile([128, 52], F32, name="e2t")
        ec2 = sb.tile([128, 1], F32, name="ec2")
        nc.scalar.activation(out=e2t[:], in_=lgb[:], func=AF.Exp,
                             bias=negm2[:, :1], accum_out=ec2[:, :1])
        z2r = sb.tile([128, 1], F32, name="z2r")
        nc.gpsimd.partition_all_reduce(z2r[:], ec2[:], channels=128,
                                       reduce_op=bass_isa.ReduceOp.add)
        pay2 = sb.tile([1, 16], F32, name="pay2")
        nc.vector.memset(pay2[:], 0.0)
        nc.vector.tensor_copy(out=pay2[0:1, 0:1], in_=m2r[0:1, :])
        nc.vector.tensor_copy(out=pay2[0:1, 1:2], in_=z2r[0:1, :])
        pay2_dr = dram.tile([16], F32, name="pay2_dr")
        nc.sync.dma_start(out=pay2_dr[None, :], in_=pay2[:])
        p28_dr = dram.tile([C * 16], F32, name="p28_dr", addr_space="Shared")
        nc.gpsimd.collective_compute(
            "AllGather", mybir.AluOpType.bypass, replica_groups=RG,
            ins=[pay2_dr[:].opt()], outs=[p28_dr[:].opt()])
        p28 = sb.tile([8, 16], F32, name="p28")
        nc.sync.dma_start(out=p28[:], in_=p28_dr[:].rearrange("(a b) -> a b", a=8))
        m28 = p28[:, 0:1]
        z28 = p28[:, 1:2]
        mg2 = sb.tile([8, 1], F32, name="mg2")
        nc.gpsimd.partition_all_reduce(mg2[:], m28, channels=8,
                                       reduce_op=bass_isa.ReduceOp.max)
        ng2 = sb.tile([8, 1], F32, name="ng2")
        nc.vector.tensor_scalar_mul(ng2[:], mg2[:], -1.0)
        t28 = sb.tile([8, 1], F32, name="t28")
        nc.scalar.activation(out=t28[:], in_=m28, func=AF.Exp, bias=ng2[:, :1])
        u28 = sb.tile([8, 1], F32, name="u28")
        nc.vector.tensor_mul(out=u28[:], in0=z28, in1=t28[:])
        zg2 = sb.tile([8, 1], F32, name="zg2")
        nc.gpsimd.partition_all_reduce(zg2[:], u28[:], channels=8,
                                       reduce_op=bass_isa.ReduceOp.add)
        l8 = sb.tile([8, 1], F32, name="l8")
        nc.scalar.activation(out=l8[:], in_=zg2[:], func=AF.Ln)
        lse8 = sb.tile([8, 1], F32, name="lse8")
        nc.vector.tensor_add(out=lse8[:], in0=l8[:], in1=mg2[:])
        nls8 = sb.tile([8, 1], F32, name="nls8")
        nc.vector.tensor_scalar_mul(nls8[:], lse8[:], -1.0)
        nls = sb.tile([128, 1], F32, name="nls")
        nc.gpsimd.partition_broadcast(nls[:], nls8[0:1, :])
        outv = sb.tile([128, 52], F32, name="outv")
        nc.scalar.activation(out=outv[:], in_=lgb[:], func=AF.Identity,
                             bias=nls[:, :1])
        nc.sync.dma_start(out=o_logits[:].rearrange("(p f) -> p f", p=128),
                          in_=outv[:])

    nc.compile()
    return nc


def _prep_inputs(input, hidden, encoder_hiddens, embedding, w_ih, w_hh,
                 b_ih, b_hh, out_w, out_b):
    token = int(np.asarray(input).ravel()[0])
    hidden = np.asarray(hidden, dtype=np.float32).reshape(H)
    encoder_hiddens = np.asarray(encoder_hiddens, dtype=np.float32)
    embedding = np.asarray(embedding, dtype=np.float32)
    w_ih = np.asarray(w_ih, dtype=np.float32)
    w_hh = np.asarray(w_hh, dtype=np.float32)
    b_ih = np.asarray(b_ih, dtype=np.float32)
    b_hh = np.asarray(b_hh, dtype=np.float32)
    out_w = np.asarray(out_w, dtype=np.float32)
    out_b = np.asarray(out_b, dtype=np.float32)

    epad = np.vstack([embedding, np.zeros((C * ES - V, H), np.float32)])
    wpad = np.vstack([out_w, np.zeros((C * S - V, 2 * H), np.float32)])
    bpad = np.concatenate([out_b, np.full(C * S - V, -1e30, np.float32)])

    h8 = hidden.reshape(8, 128)
    in_maps = []
    for c in range(C):
        hs = slice(c * 128, (c + 1) * 128)
        # GRU weight shards: rows ordered [r;z;n] for this core's h slice
        rows = np.concatenate([np.arange(g * H + c * 128, g * H + (c + 1) * 128)
                               for g in range(3)])
        Rih = w_ih[rows]                       # [384, 1024]
        Rhh = w_hh[rows]
        wih_lay = np.ascontiguousarray(
            Rih.T.reshape(8, 128, 3, 128).transpose(1, 0, 2, 3))
        whh_lay = np.ascontiguousarray(
            Rhh.T.reshape(8, 128, 3, 128).transpose(1, 0, 2, 3))
        bih_lay = np.ascontiguousarray(b_ih[rows].reshape(3, 128).T)
        bhh_lay = np.ascontiguousarray(b_hh[rows].reshape(3, 128).T)

        eh_c = np.ascontiguousarray(
            encoder_hiddens[c * 512:(c + 1) * 512].reshape(LT, 128, H))

        B = wpad[c * S:(c + 1) * S]            # [S, 2048]
        wt_lay = np.ascontiguousarray(
            B.T.reshape(2, 8, 128, NT, 512).transpose(3, 0, 2, 1, 4)
        ).astype(ml_dtypes.bfloat16)
        blay = np.ascontiguousarray(bpad[c * S:(c + 1) * S].reshape(128, 52))

        etab_c = np.ascontiguousarray(
            epad[c * ES:(c + 1) * ES].reshape(2 * ES, 512))

        in_maps.append(dict(
            tok=np.array([[token]], np.int32),
            vbase=np.array([[c * ES]], np.int32),
            etab=etab_c,
            wih=wih_lay, whh=whh_lay, bih=bih_lay, bhh=bhh_lay,
            hsh=np.ascontiguousarray(hidden[hs]),
            h8in=h8,
            eh=eh_c,
            wt=wt_lay,
            blay=blay,
        ))
    return in_maps


def kernel(**inputs):
    if "nc" not in _CACHE:
        _CACHE["nc"] = _build()
    nc = _CACHE["nc"]
    in_maps = _prep_inputs(**inputs)
    res = run_bass_kernel_spmd(nc, in_maps, list(range(C)))
    logits = np.concatenate([res.results[c]["o_logits"] for c in range(C)])[:V]
    h_new = res.results[0]["o_hnew"]
    return logits.reshape(1, V), h_new.reshape(1, 1, H)
